# revision 7
# baseline (speedup 1.0000x reference)
import numpy as np
import concourse.bacc as bacc
import concourse.mybir as mybir
from concourse.tile import TileContext
from concourse.bass_utils import run_bass_kernel_spmd

DIM_INPUT = 128
DIM_REC = 512
DIM_OUT = 256
BATCH = 512
NCORES = 8
B = BATCH // NCORES  # 64 per-core batch
T = DIM_INPUT        # 128 timesteps
KJ = DIM_REC // 128  # 4 chunks of the recurrent dim
OJ = DIM_OUT // 128  # 2 chunks of the output dim

F32 = mybir.dt.float32

# MM issue order within a step: (j, k) pairs chosen so that the producer
# group of g'_k finishes as many slots as possible before the next step's
# first consumer of g'_k (min slack 7 of 16 slots).
STEP_ORDER = [
    (0, 0), (1, 0), (2, 0), (3, 0),
    (0, 1), (0, 2), (0, 3),
    (1, 1), (1, 2), (1, 3),
    (2, 1), (2, 2), (2, 3),
    (3, 1), (3, 2), (3, 3),
]


def _build_nc():
    nc = bacc.Bacc("TRN2", target_bir_lowering=False, debug=False,
                   num_devices=NCORES)
    xT = nc.dram_tensor("xT", [DIM_INPUT, B], F32, kind="ExternalInput")
    WhT = nc.dram_tensor("WhT", [DIM_REC, DIM_REC], F32, kind="ExternalInput")
    WxT = nc.dram_tensor("WxT", [DIM_INPUT, DIM_REC], F32, kind="ExternalInput")
    WhyT = nc.dram_tensor("WhyT", [DIM_REC, DIM_OUT], F32, kind="ExternalInput")
    bc = nc.dram_tensor("bc", [DIM_REC, 1], F32, kind="ExternalInput")
    by = nc.dram_tensor("by", [DIM_OUT, 1], F32, kind="ExternalInput")
    yT = nc.dram_tensor("yT", [DIM_OUT, B], F32, kind="ExternalOutput")

    RELU = mybir.ActivationFunctionType.Relu
    IDENT = mybir.ActivationFunctionType.Identity

    with TileContext(nc) as tc:
        with tc.tile_pool(name="w", bufs=1) as wp, \
             tc.tile_pool(name="s", bufs=1) as sp, \
             tc.psum_pool(name="p", bufs=1) as pp:
            wh = [wp.tile([128, DIM_REC], F32, name=f"wh{k}") for k in range(KJ)]
            wx = wp.tile([128, DIM_REC], F32, name="wx")
            why = [wp.tile([128, DIM_OUT], F32, name=f"why{k}") for k in range(KJ)]
            bct = [wp.tile([128, 1], F32, name=f"bct{k}") for k in range(KJ)]
            byt = [wp.tile([128, 1], F32, name=f"byt{j}") for j in range(OJ)]
            xt = sp.tile([128, B], F32, name="xt")
            c = [sp.tile([128, B], F32, name=f"c{j}") for j in range(KJ)]
            g = [[sp.tile([128, B], F32, name=f"g{p}_{k}") for k in range(KJ)]
                 for p in range(2)]
            ps = [[pp.tile([128, B], F32, name=f"ps{p}_{j}") for j in range(KJ)]
                  for p in range(2)]
            psy = [ps[0][0], ps[0][1]]  # reuse phase-0 banks (free after step T-1)

            for k in range(KJ):
                nc.sync.dma_start(out=wh[k][:], in_=WhT[k * 128:(k + 1) * 128, :])
                nc.sync.dma_start(out=why[k][:], in_=WhyT[k * 128:(k + 1) * 128, :])
                nc.sync.dma_start(out=bct[k][:], in_=bc[k * 128:(k + 1) * 128, :])
            for j in range(OJ):
                nc.sync.dma_start(out=byt[j][:], in_=by[j * 128:(j + 1) * 128, :])
            nc.sync.dma_start(out=wx[:], in_=WxT[:])
            nc.sync.dma_start(out=xt[:], in_=xT[:])

            # c_j = (x @ W_x2h.T).T[jslice] + (b_x2h + b_h2h)[jslice]
            # g0_j = relu(c_j)  (step 1: h0 = 0)
            for j in range(KJ):
                nc.tensor.matmul(ps[0][j][:], wx[:, j * 128:(j + 1) * 128],
                                 xt[:], start=True, stop=True)
            for j in range(KJ):
                nc.scalar.activation(c[j][:], ps[0][j][:], IDENT, bias=bct[j][:])
                nc.scalar.activation(g[0][j][:], ps[0][j][:], RELU, bias=bct[j][:])

            # 127 recurrent steps: g' = relu(c + Wh @ g)
            for s in range(1, T):
                cur, nxt = g[(s + 1) % 2], g[s % 2]
                pcur = ps[s % 2]
                grp = [0] * KJ
                for (j, k) in STEP_ORDER:
                    nc.tensor.matmul(pcur[j][:], wh[k][:, j * 128:(j + 1) * 128],
                                     cur[k][:], start=(grp[j] == 0),
                                     stop=(grp[j] == KJ - 1))
                    grp[j] += 1
                for j in range(KJ):
                    nc.vector.tensor_add(nxt[j][:], pcur[j][:], c[j][:])
                    nc.scalar.activation(nxt[j][:], nxt[j][:], RELU)

            gfin = g[(T - 1) % 2]
            # yT[jslice] = W_h2y[jslice] @ h.T + b_h2y[jslice]
            for j in range(OJ):
                for k in range(KJ):
                    nc.tensor.matmul(psy[j][:], why[k][:, j * 128:(j + 1) * 128],
                                     gfin[k][:], start=(k == 0), stop=(k == KJ - 1))
            ytile = [sp.tile([128, B], F32, name=f"yt{j}") for j in range(OJ)]
            for j in range(OJ):
                nc.scalar.activation(ytile[j][:], psy[j][:], IDENT, bias=byt[j][:])
                nc.sync.dma_start(out=yT[j * 128:(j + 1) * 128, :], in_=ytile[j][:])

    nc.compile()
    return nc


_NC = None
TRACE = False
TRACE_TMPDIR = None
LAST_RESULTS = None


def kernel(x, W_x2h, b_x2h, W_h2h, b_h2h, W_h2y, b_h2y):
    global _NC, LAST_RESULTS
    if _NC is None:
        _NC = _build_nc()

    x = np.asarray(x, np.float32)
    shared = {
        "WhT": np.ascontiguousarray(np.asarray(W_h2h, np.float32).T),
        "WxT": np.ascontiguousarray(np.asarray(W_x2h, np.float32).T),
        "WhyT": np.ascontiguousarray(np.asarray(W_h2y, np.float32).T),
        "bc": (np.asarray(b_x2h, np.float32)
               + np.asarray(b_h2h, np.float32)).reshape(DIM_REC, 1),
        "by": np.asarray(b_h2y, np.float32).reshape(DIM_OUT, 1),
    }
    ins = []
    for i in range(NCORES):
        m = dict(shared)
        m["xT"] = np.ascontiguousarray(x[i * B:(i + 1) * B, :].T)
        ins.append(m)

    kw = {}
    if TRACE:
        kw = {"trace": True, "tmpdir": TRACE_TMPDIR}
    res = run_bass_kernel_spmd(_NC, ins, core_ids=list(range(NCORES)), **kw)
    LAST_RESULTS = res
    out = np.empty((BATCH, DIM_OUT), np.float32)
    for i in range(NCORES):
        out[i * B:(i + 1) * B, :] = res.results[i]["yT"].T
    return out


# revision 10
# speedup vs baseline: 1.9406x; 1.9406x over previous
import numpy as np
import concourse.bacc as bacc
import concourse.mybir as mybir
from concourse.tile import TileContext
from concourse.bass_utils import run_bass_kernel_spmd

DIM_INPUT = 128
DIM_REC = 512
DIM_OUT = 256
BATCH = 512
NCORES = 8
B = BATCH // NCORES  # 64 per-core batch
T = DIM_INPUT        # 128 timesteps
KJ = DIM_REC // 128  # 4 chunks of the recurrent dim
OJ = DIM_OUT // 128  # 2 chunks of the output dim

F32 = mybir.dt.float32
MMDT = mybir.dt.float32r  # matmul operand dtype (single-pass PE, ~tf32+)

# MM issue order within a step: (j, k) pairs chosen so that the producer
# group of g'_k finishes as many slots as possible before the next step's
# first consumer of g'_k (min slack 7 of 16 slots).
STEP_ORDER = [
    (0, 0), (1, 0), (2, 0), (3, 0),
    (0, 1), (0, 2), (0, 3),
    (1, 1), (1, 2), (1, 3),
    (2, 1), (2, 2), (2, 3),
    (3, 1), (3, 2), (3, 3),
]


def _build_nc():
    nc = bacc.Bacc("TRN2", target_bir_lowering=False, debug=False,
                   num_devices=NCORES)
    xT = nc.dram_tensor("xT", [DIM_INPUT, B], MMDT, kind="ExternalInput")
    WhT = nc.dram_tensor("WhT", [DIM_REC, DIM_REC], MMDT, kind="ExternalInput")
    WxT = nc.dram_tensor("WxT", [DIM_INPUT, DIM_REC], MMDT, kind="ExternalInput")
    WhyT = nc.dram_tensor("WhyT", [DIM_REC, DIM_OUT], MMDT, kind="ExternalInput")
    bc = nc.dram_tensor("bc", [DIM_REC, 1], F32, kind="ExternalInput")
    by = nc.dram_tensor("by", [DIM_OUT, 1], F32, kind="ExternalInput")
    yT = nc.dram_tensor("yT", [DIM_OUT, B], F32, kind="ExternalOutput")

    RELU = mybir.ActivationFunctionType.Relu
    IDENT = mybir.ActivationFunctionType.Identity

    with TileContext(nc) as tc:
        with tc.tile_pool(name="w", bufs=1) as wp, \
             tc.tile_pool(name="s", bufs=1) as sp, \
             tc.psum_pool(name="p", bufs=1) as pp:
            wh = [wp.tile([128, DIM_REC], MMDT, name=f"wh{k}") for k in range(KJ)]
            wx = wp.tile([128, DIM_REC], MMDT, name="wx")
            why = [wp.tile([128, DIM_OUT], MMDT, name=f"why{k}") for k in range(KJ)]
            bct = [wp.tile([128, 1], F32, name=f"bct{k}") for k in range(KJ)]
            byt = [wp.tile([128, 1], F32, name=f"byt{j}") for j in range(OJ)]
            xt = sp.tile([128, B], MMDT, name="xt")
            c = [sp.tile([128, B], F32, name=f"c{j}") for j in range(KJ)]
            g = [[sp.tile([128, B], MMDT, name=f"g{p}_{k}") for k in range(KJ)]
                 for p in range(2)]
            ps = [[pp.tile([128, B], F32, name=f"ps{p}_{j}") for j in range(KJ)]
                  for p in range(2)]
            psy = [ps[0][0], ps[0][1]]  # reuse phase-0 banks (free after step T-1)

            for k in range(KJ):
                nc.sync.dma_start(out=wh[k][:], in_=WhT[k * 128:(k + 1) * 128, :])
                nc.sync.dma_start(out=why[k][:], in_=WhyT[k * 128:(k + 1) * 128, :])
                nc.sync.dma_start(out=bct[k][:], in_=bc[k * 128:(k + 1) * 128, :])
            for j in range(OJ):
                nc.sync.dma_start(out=byt[j][:], in_=by[j * 128:(j + 1) * 128, :])
            nc.sync.dma_start(out=wx[:], in_=WxT[:])
            nc.sync.dma_start(out=xt[:], in_=xT[:])

            # c_j = (x @ W_x2h.T).T[jslice] + (b_x2h + b_h2h)[jslice]
            # g0_j = relu(c_j)  (step 1: h0 = 0)
            for j in range(KJ):
                nc.tensor.matmul(ps[0][j][:], wx[:, j * 128:(j + 1) * 128],
                                 xt[:], start=True, stop=True)
            for j in range(KJ):
                nc.scalar.activation(c[j][:], ps[0][j][:], IDENT, bias=bct[j][:])
                nc.scalar.activation(g[0][j][:], ps[0][j][:], RELU, bias=bct[j][:])

            # 127 recurrent steps: g' = relu(c + Wh @ g)
            for s in range(1, T):
                cur, nxt = g[(s + 1) % 2], g[s % 2]
                pcur = ps[s % 2]
                grp = [0] * KJ
                for (j, k) in STEP_ORDER:
                    nc.tensor.matmul(pcur[j][:], wh[k][:, j * 128:(j + 1) * 128],
                                     cur[k][:], start=(grp[j] == 0),
                                     stop=(grp[j] == KJ - 1))
                    grp[j] += 1
                for j in range(KJ):
                    nc.vector.tensor_add(nxt[j][:], pcur[j][:], c[j][:])
                    nc.scalar.activation(nxt[j][:], nxt[j][:], RELU)

            gfin = g[(T - 1) % 2]
            # yT[jslice] = W_h2y[jslice] @ h.T + b_h2y[jslice]
            for j in range(OJ):
                for k in range(KJ):
                    nc.tensor.matmul(psy[j][:], why[k][:, j * 128:(j + 1) * 128],
                                     gfin[k][:], start=(k == 0), stop=(k == KJ - 1))
            ytile = [sp.tile([128, B], F32, name=f"yt{j}") for j in range(OJ)]
            for j in range(OJ):
                nc.scalar.activation(ytile[j][:], psy[j][:], IDENT, bias=byt[j][:])
                nc.sync.dma_start(out=yT[j * 128:(j + 1) * 128, :], in_=ytile[j][:])

    nc.compile()
    return nc


_NC = None
TRACE = False
TRACE_TMPDIR = None
LAST_RESULTS = None


def kernel(x, W_x2h, b_x2h, W_h2h, b_h2h, W_h2y, b_h2y):
    global _NC, LAST_RESULTS
    if _NC is None:
        _NC = _build_nc()

    x = np.asarray(x, np.float32)
    shared = {
        "WhT": np.ascontiguousarray(np.asarray(W_h2h, np.float32).T),
        "WxT": np.ascontiguousarray(np.asarray(W_x2h, np.float32).T),
        "WhyT": np.ascontiguousarray(np.asarray(W_h2y, np.float32).T),
        "bc": (np.asarray(b_x2h, np.float32)
               + np.asarray(b_h2h, np.float32)).reshape(DIM_REC, 1),
        "by": np.asarray(b_h2y, np.float32).reshape(DIM_OUT, 1),
    }
    ins = []
    for i in range(NCORES):
        m = dict(shared)
        m["xT"] = np.ascontiguousarray(x[i * B:(i + 1) * B, :].T)
        ins.append(m)

    kw = {}
    if TRACE:
        kw = {"trace": True, "tmpdir": TRACE_TMPDIR}
    res = run_bass_kernel_spmd(_NC, ins, core_ids=list(range(NCORES)), **kw)
    LAST_RESULTS = res
    out = np.empty((BATCH, DIM_OUT), np.float32)
    for i in range(NCORES):
        out[i * B:(i + 1) * B, :] = res.results[i]["yT"].T
    return out


# revision 12
# speedup vs baseline: 3.6641x; 1.8882x over previous
import numpy as np
import concourse.bacc as bacc
import concourse.mybir as mybir
from concourse.tile import TileContext
from concourse.bass_utils import run_bass_kernel_spmd

DIM_INPUT = 128
DIM_REC = 512
DIM_OUT = 256
BATCH = 512
NCORES = 8
B = BATCH // NCORES  # 64 per-core batch
T = DIM_INPUT        # 128 timesteps
KJ = DIM_REC // 128  # 4 chunks of the recurrent dim
OJ = DIM_OUT // 128  # 2 chunks of the output dim

F32 = mybir.dt.float32
MMDT = mybir.dt.float16  # matmul operand dtype (FWL + 1 cyc/row on PE)
MMNP = np.float16

# MM issue order within a step: (j, k) pairs chosen so that the producer
# group of g'_k finishes as many slots as possible before the next step's
# first consumer of g'_k (min slack 7 of 16 slots).
STEP_ORDER = [
    (0, 0), (1, 0), (2, 0), (3, 0),
    (0, 1), (0, 2), (0, 3),
    (1, 1), (1, 2), (1, 3),
    (2, 1), (2, 2), (2, 3),
    (3, 1), (3, 2), (3, 3),
]


def _build_nc():
    nc = bacc.Bacc("TRN2", target_bir_lowering=False, debug=False,
                   num_devices=NCORES)
    xT = nc.dram_tensor("xT", [DIM_INPUT, B], MMDT, kind="ExternalInput")
    WhT = nc.dram_tensor("WhT", [DIM_REC, DIM_REC], MMDT, kind="ExternalInput")
    WxT = nc.dram_tensor("WxT", [DIM_INPUT, DIM_REC], MMDT, kind="ExternalInput")
    WhyT = nc.dram_tensor("WhyT", [DIM_REC, DIM_OUT], MMDT, kind="ExternalInput")
    bc = nc.dram_tensor("bc", [DIM_REC, 1], F32, kind="ExternalInput")
    by = nc.dram_tensor("by", [DIM_OUT, 1], F32, kind="ExternalInput")
    yT = nc.dram_tensor("yT", [DIM_OUT, B], F32, kind="ExternalOutput")

    RELU = mybir.ActivationFunctionType.Relu
    IDENT = mybir.ActivationFunctionType.Identity

    with TileContext(nc) as tc:
        with tc.tile_pool(name="w", bufs=1) as wp, \
             tc.tile_pool(name="s", bufs=1) as sp, \
             tc.psum_pool(name="p", bufs=1) as pp:
            wh = [wp.tile([128, DIM_REC], MMDT, name=f"wh{k}") for k in range(KJ)]
            wx = wp.tile([128, DIM_REC], MMDT, name="wx")
            why = [wp.tile([128, DIM_OUT], MMDT, name=f"why{k}") for k in range(KJ)]
            bct = [wp.tile([128, 1], F32, name=f"bct{k}") for k in range(KJ)]
            byt = [wp.tile([128, 1], F32, name=f"byt{j}") for j in range(OJ)]
            xt = sp.tile([128, B], MMDT, name="xt")
            c = [sp.tile([128, B], F32, name=f"c{j}") for j in range(KJ)]
            g = [[sp.tile([128, B], MMDT, name=f"g{p}_{k}") for k in range(KJ)]
                 for p in range(2)]
            ps = [[pp.tile([128, B], F32, name=f"ps{p}_{j}") for j in range(KJ)]
                  for p in range(2)]
            psy = [ps[0][0], ps[0][1]]  # reuse phase-0 banks (free after step T-1)

            for k in range(KJ):
                nc.sync.dma_start(out=wh[k][:], in_=WhT[k * 128:(k + 1) * 128, :])
                nc.sync.dma_start(out=why[k][:], in_=WhyT[k * 128:(k + 1) * 128, :])
                nc.sync.dma_start(out=bct[k][:], in_=bc[k * 128:(k + 1) * 128, :])
            for j in range(OJ):
                nc.sync.dma_start(out=byt[j][:], in_=by[j * 128:(j + 1) * 128, :])
            nc.sync.dma_start(out=wx[:], in_=WxT[:])
            nc.sync.dma_start(out=xt[:], in_=xT[:])

            # c_j = (x @ W_x2h.T).T[jslice] + (b_x2h + b_h2h)[jslice]
            # g0_j = relu(c_j)  (step 1: h0 = 0)
            for j in range(KJ):
                nc.tensor.matmul(ps[0][j][:], wx[:, j * 128:(j + 1) * 128],
                                 xt[:], start=True, stop=True)
            for j in range(KJ):
                nc.scalar.activation(c[j][:], ps[0][j][:], IDENT, bias=bct[j][:])
                nc.scalar.activation(g[0][j][:], ps[0][j][:], RELU, bias=bct[j][:])

            # 127 recurrent steps: g' = relu(c + Wh @ g)
            for s in range(1, T):
                cur, nxt = g[(s + 1) % 2], g[s % 2]
                pcur = ps[s % 2]
                grp = [0] * KJ
                for (j, k) in STEP_ORDER:
                    nc.tensor.matmul(pcur[j][:], wh[k][:, j * 128:(j + 1) * 128],
                                     cur[k][:], start=(grp[j] == 0),
                                     stop=(grp[j] == KJ - 1))
                    grp[j] += 1
                for j in range(KJ):
                    nc.vector.tensor_add(nxt[j][:], pcur[j][:], c[j][:])
                    nc.scalar.activation(nxt[j][:], nxt[j][:], RELU)

            gfin = g[(T - 1) % 2]
            # yT[jslice] = W_h2y[jslice] @ h.T + b_h2y[jslice]
            for j in range(OJ):
                for k in range(KJ):
                    nc.tensor.matmul(psy[j][:], why[k][:, j * 128:(j + 1) * 128],
                                     gfin[k][:], start=(k == 0), stop=(k == KJ - 1))
            ytile = [sp.tile([128, B], F32, name=f"yt{j}") for j in range(OJ)]
            for j in range(OJ):
                nc.scalar.activation(ytile[j][:], psy[j][:], IDENT, bias=byt[j][:])
                nc.sync.dma_start(out=yT[j * 128:(j + 1) * 128, :], in_=ytile[j][:])

    nc.compile()
    return nc


_NC = None
TRACE = False
TRACE_TMPDIR = None
LAST_RESULTS = None


def kernel(x, W_x2h, b_x2h, W_h2h, b_h2h, W_h2y, b_h2y):
    global _NC, LAST_RESULTS
    if _NC is None:
        _NC = _build_nc()

    x = np.asarray(x, np.float32)
    shared = {
        "WhT": np.ascontiguousarray(np.asarray(W_h2h, np.float32).T.astype(MMNP)),
        "WxT": np.ascontiguousarray(np.asarray(W_x2h, np.float32).T.astype(MMNP)),
        "WhyT": np.ascontiguousarray(np.asarray(W_h2y, np.float32).T.astype(MMNP)),
        "bc": (np.asarray(b_x2h, np.float32)
               + np.asarray(b_h2h, np.float32)).reshape(DIM_REC, 1),
        "by": np.asarray(b_h2y, np.float32).reshape(DIM_OUT, 1),
    }
    ins = []
    for i in range(NCORES):
        m = dict(shared)
        m["xT"] = np.ascontiguousarray(x[i * B:(i + 1) * B, :].T.astype(MMNP))
        ins.append(m)

    kw = {}
    if TRACE:
        kw = {"trace": True, "tmpdir": TRACE_TMPDIR}
    res = run_bass_kernel_spmd(_NC, ins, core_ids=list(range(NCORES)), **kw)
    LAST_RESULTS = res
    out = np.empty((BATCH, DIM_OUT), np.float32)
    for i in range(NCORES):
        out[i * B:(i + 1) * B, :] = res.results[i]["yT"].T
    return out


# revision 15
# speedup vs baseline: 5.8708x; 1.6022x over previous
import numpy as np
import concourse.bacc as bacc
import concourse.mybir as mybir
from concourse.tile import TileContext
from concourse.bass_utils import run_bass_kernel_spmd

DIM_INPUT = 128
DIM_REC = 512
DIM_OUT = 256
BATCH = 512
NCORES = 8
B = BATCH // NCORES  # 64 per-core batch
T = DIM_INPUT        # 128 timesteps
KJ = DIM_REC // 128  # 4 chunks of the recurrent dim
OJ = DIM_OUT // 128  # 2 chunks of the output dim

F32 = mybir.dt.float32
MMDT = mybir.dt.float16  # matmul operand dtype (FWL + 1 cyc/row on PE)
MMNP = np.float16

# MM issue order within a step. Each output group j accumulates 5 MMs:
# an x-projection MM ('x', start=True: psum = x @ Wx[j]) plus 4 recurrent
# MMs (k=0..3). Recomputing the x MM each step seeds psum so the epilogue
# is a single fused bias+relu per group. The (j,k) suborder maximizes the
# min slack between group-k completion and the next step's first consumer
# of g'_k (slack 12 of 20 slots; >=13 provably infeasible).
STEP_ORDER = [
    (0, 'x'), (1, 'x'), (2, 'x'), (3, 'x'),
    (0, 0), (1, 0), (2, 0), (3, 0),
    (0, 1), (1, 1), (2, 1),
    (0, 2), (0, 3),
    (1, 2), (1, 3),
    (3, 1),
    (2, 2), (2, 3),
    (3, 2), (3, 3),
]


def _build_nc():
    nc = bacc.Bacc("TRN2", target_bir_lowering=False, debug=False,
                   num_devices=NCORES)
    xT = nc.dram_tensor("xT", [DIM_INPUT, B], MMDT, kind="ExternalInput")
    WhT = nc.dram_tensor("WhT", [DIM_REC, DIM_REC], MMDT, kind="ExternalInput")
    WxT = nc.dram_tensor("WxT", [DIM_INPUT, DIM_REC], MMDT, kind="ExternalInput")
    WhyT = nc.dram_tensor("WhyT", [DIM_REC, DIM_OUT], MMDT, kind="ExternalInput")
    bc = nc.dram_tensor("bc", [DIM_REC, 1], F32, kind="ExternalInput")
    by = nc.dram_tensor("by", [DIM_OUT, 1], F32, kind="ExternalInput")
    yT = nc.dram_tensor("yT", [DIM_OUT, B], F32, kind="ExternalOutput")

    RELU = mybir.ActivationFunctionType.Relu
    IDENT = mybir.ActivationFunctionType.Identity

    with TileContext(nc) as tc:
        with tc.tile_pool(name="w", bufs=1) as wp, \
             tc.tile_pool(name="s", bufs=1) as sp, \
             tc.psum_pool(name="p", bufs=1) as pp:
            wh = [wp.tile([128, DIM_REC], MMDT, name=f"wh{k}") for k in range(KJ)]
            wx = wp.tile([128, DIM_REC], MMDT, name="wx")
            why = [wp.tile([128, DIM_OUT], MMDT, name=f"why{k}") for k in range(KJ)]
            bct = [wp.tile([128, 1], F32, name=f"bct{k}") for k in range(KJ)]
            byt = [wp.tile([128, 1], F32, name=f"byt{j}") for j in range(OJ)]
            xt = sp.tile([128, B], MMDT, name="xt")
            g = [[sp.tile([128, B], MMDT, name=f"g{p}_{k}") for k in range(KJ)]
                 for p in range(2)]
            ps = [[pp.tile([128, B], F32, name=f"ps{p}_{j}") for j in range(KJ)]
                  for p in range(2)]
            psy = [ps[0][0], ps[0][1]]  # reuse phase-0 banks (free after step T-1)

            for k in range(KJ):
                nc.sync.dma_start(out=wh[k][:], in_=WhT[k * 128:(k + 1) * 128, :])
                nc.sync.dma_start(out=why[k][:], in_=WhyT[k * 128:(k + 1) * 128, :])
                nc.sync.dma_start(out=bct[k][:], in_=bc[k * 128:(k + 1) * 128, :])
            for j in range(OJ):
                nc.sync.dma_start(out=byt[j][:], in_=by[j * 128:(j + 1) * 128, :])
            nc.sync.dma_start(out=wx[:], in_=WxT[:])
            nc.sync.dma_start(out=xt[:], in_=xT[:])

            ADD = mybir.AluOpType.add
            MAX = mybir.AluOpType.max

            def epilogue(dst, psrc):
                # dst_j = relu(psum_j + bc_j); 2 groups on ScalarE, 2 on DVE
                for j in range(2):
                    nc.scalar.activation(dst[j][:], psrc[j][:], RELU,
                                         bias=bct[j][:])
                for j in range(2, KJ):
                    nc.vector.tensor_scalar(dst[j][:], psrc[j][:],
                                            bct[j][:], 0.0, ADD, MAX)

            # step 1 (h0 = 0): g0_j = relu((x @ W_x2h.T).T[j] + bc[j])
            for j in range(KJ):
                nc.tensor.matmul(ps[0][j][:], wx[:, j * 128:(j + 1) * 128],
                                 xt[:], start=True, stop=True)
            epilogue(g[0], ps[0])

            # 127 recurrent steps: g' = relu(x @ Wx + Wh @ g + bc)
            for s in range(1, T):
                cur, nxt = g[(s + 1) % 2], g[s % 2]
                pcur = ps[s % 2]
                grp = [0] * KJ
                for (j, k) in STEP_ORDER:
                    if k == 'x':
                        nc.tensor.matmul(pcur[j][:],
                                         wx[:, j * 128:(j + 1) * 128],
                                         xt[:], start=True, stop=False)
                    else:
                        nc.tensor.matmul(pcur[j][:],
                                         wh[k][:, j * 128:(j + 1) * 128],
                                         cur[k][:], start=False,
                                         stop=(grp[j] == KJ - 1))
                        grp[j] += 1
                epilogue(nxt, pcur)

            gfin = g[(T - 1) % 2]
            # yT[jslice] = W_h2y[jslice] @ h.T + b_h2y[jslice]
            for j in range(OJ):
                for k in range(KJ):
                    nc.tensor.matmul(psy[j][:], why[k][:, j * 128:(j + 1) * 128],
                                     gfin[k][:], start=(k == 0), stop=(k == KJ - 1))
            ytile = [sp.tile([128, B], F32, name=f"yt{j}") for j in range(OJ)]
            for j in range(OJ):
                nc.scalar.activation(ytile[j][:], psy[j][:], IDENT, bias=byt[j][:])
                nc.sync.dma_start(out=yT[j * 128:(j + 1) * 128, :], in_=ytile[j][:])

    nc.compile()
    return nc


_NC = None
TRACE = False
TRACE_TMPDIR = None
LAST_RESULTS = None


def kernel(x, W_x2h, b_x2h, W_h2h, b_h2h, W_h2y, b_h2y):
    global _NC, LAST_RESULTS
    if _NC is None:
        _NC = _build_nc()

    x = np.asarray(x, np.float32)
    shared = {
        "WhT": np.ascontiguousarray(np.asarray(W_h2h, np.float32).T.astype(MMNP)),
        "WxT": np.ascontiguousarray(np.asarray(W_x2h, np.float32).T.astype(MMNP)),
        "WhyT": np.ascontiguousarray(np.asarray(W_h2y, np.float32).T.astype(MMNP)),
        "bc": (np.asarray(b_x2h, np.float32)
               + np.asarray(b_h2h, np.float32)).reshape(DIM_REC, 1),
        "by": np.asarray(b_h2y, np.float32).reshape(DIM_OUT, 1),
    }
    ins = []
    for i in range(NCORES):
        m = dict(shared)
        m["xT"] = np.ascontiguousarray(x[i * B:(i + 1) * B, :].T.astype(MMNP))
        ins.append(m)

    kw = {}
    if TRACE:
        kw = {"trace": True, "tmpdir": TRACE_TMPDIR}
    res = run_bass_kernel_spmd(_NC, ins, core_ids=list(range(NCORES)), **kw)
    LAST_RESULTS = res
    out = np.empty((BATCH, DIM_OUT), np.float32)
    for i in range(NCORES):
        out[i * B:(i + 1) * B, :] = res.results[i]["yT"].T
    return out
